# revision 28
# baseline (speedup 1.0000x reference)
"""Trainium2 Bass kernel for nn_Attention (GroupNorm + 1x1-conv QKV + MHA + out-proj + residual).

Sharding: data-parallel over batch — b=8 maps one batch element per NeuronCore (8 cores).
Weights are broadcast to all cores. No collectives.

v2 design (ACT-bound pipeline, target ~ the 64-instruction exp stream):
  - GroupNorm(32 groups): per-channel sum on Pool, sum-of-squares on DVE
    (tensor_tensor_reduce), group-reduce / broadcast via tiny group-indicator
    matmuls; rsqrt(var+eps) = Exp(-0.5*Ln(var+eps)) on ACT so the only
    activation table ever loaded is the exp+ln set (no table swaps).
    Apply h = x*A + B as one DVE tensor_scalar per 128-ch chunk -> bf16.
  - Q/K: bf16 matmuls (full-rate PE), bias added during the PSUM->SBUF move
    on DVE (bf16 out).
  - vT computed directly as h^T Wv^T in bf16; cast to fp8e4 during the
    PSUM->SBUF copy into the stationary vtt tile ([vT_A|ones|ones|vT_B] per
    head pair; ones columns loaded once from DRAM). The v bias is folded
    into the out-projection bias on the host (softmax rows sum to 1).
  - Attention per head (d=64, n=1024), softmax-transposed layout:
      simT[j,i] = k^T q          (bf16, K=64)
      E = exp(simT*scale - 2.5)  (ACT, PSUM->SBUF, fp8e4 out into the
                                  per-head Et[128, 8, 1024] tile; the -2.5
                                  bias keeps E <= ~120 inside fp8e4 range
                                  and cancels in the softmax ratio)
      out[128,i] accumulated over j-chunk PAIRS with fp8 DoubleRow matmuls
      (2x PE rate): lhsT = vtt[:, 2r:2r+2, csl], rhs = Et[:, 2r:2r+2, isl].
      psum rows give numerator (64) + denominator (64 copies) via the ones
      columns at zero extra PE cost.
      att = numerator/denominator: single DVE tensor_tensor divide -> bf16
      (USE_DIVIDE=False falls back to reciprocal+mult).
  - Out-proj + bias + residual: bf16 matmuls + one DVE scalar_tensor_tensor
    epilogue per chunk; DMA out.

All input DMAs go through nc.sync (HWDGE — no engine-blocking descriptor
generation on compute engines). Weights are bf16 (half the load traffic).

chain=K builds K dependent copies of the pipeline bounced through an internal
DRAM buffer (out_i -> x_{i+1}) for slope-based hardware timing.
"""

import os
import sys

if "/opt/trn_rl_repo" not in sys.path:
    sys.path.insert(0, "/opt/trn_rl_repo")
os.environ.setdefault("JAX_PLATFORMS", "axon,cpu")

import numpy as np

B = 8
C = 512
N = 1024
HEADS = 8
DH = 64
GROUPS = 32
QKV_DIM = 3 * C
EPS = 1e-6
SCALE = DH ** -0.5  # 0.125
EBIAS = 2.5
N_CORES = 8
USE_DIVIDE = False  # DVE cannot read two PSUM operands (NCC_IBVF028)

_CACHE = {}


def _build(chain=1):
    import concourse.bacc as bacc
    import concourse.tile as tile
    from concourse import mybir
    import concourse.bass as bass
    from contextlib import ExitStack

    f32 = mybir.dt.float32
    bf16 = mybir.dt.bfloat16
    f8 = mybir.dt.float8e4
    AF = mybir.ActivationFunctionType
    OP = mybir.AluOpType
    AX = mybir.AxisListType
    DR = mybir.MatmulPerfMode.DoubleRow

    nc = bacc.Bacc("TRN2", target_bir_lowering=False, debug=False,
                   num_devices=N_CORES)

    x_d = nc.dram_tensor("x", [C, N], f32, kind="ExternalInput").ap()
    wq_d = nc.dram_tensor("w_qkvT", [C, QKV_DIM], bf16, kind="ExternalInput").ap()
    wo_d = nc.dram_tensor("w_outT", [C, C], bf16, kind="ExternalInput").ap()
    bq_d = nc.dram_tensor("b_qkvT", [128, 8], f32, kind="ExternalInput").ap()
    bo_d = nc.dram_tensor("b_outT", [128, 4], f32, kind="ExternalInput").ap()
    gam_d = nc.dram_tensor("gammaT", [128, 4], f32, kind="ExternalInput").ap()
    bet_d = nc.dram_tensor("betaT", [128, 4], f32, kind="ExternalInput").ap()
    gh_d = nc.dram_tensor("Ghat", [C, GROUPS], f32, kind="ExternalInput").ap()
    gt_d = nc.dram_tensor("GT", [GROUPS, C], f32, kind="ExternalInput").ap()
    out_d = nc.dram_tensor("out", [C, N], f32, kind="ExternalOutput").ap()
    bounce = nc.dram_tensor("chainbuf", [C, N], f32).ap() if chain > 1 else None

    with tile.TileContext(nc) as tc:
        with ExitStack() as ctx, nc.allow_low_precision(
                reason="bf16/fp8 matmul pipeline validated against reference"):
            pers = ctx.enter_context(tc.tile_pool(name="pers", bufs=1))
            consts = ctx.enter_context(tc.tile_pool(name="consts", bufs=1))
            ep = ctx.enter_context(tc.tile_pool(name="ep", bufs=2))
            vtp = ctx.enter_context(tc.tile_pool(name="vtp", bufs=2))
            smalls = ctx.enter_context(tc.tile_pool(name="smalls", bufs=4))
            scrp = ctx.enter_context(tc.tile_pool(name="scrp", bufs=2))
            # PSUM: simp 3x[128,1024] (6 banks) + poutp 1x[128,1024] (2 banks)
            # = 8 banks. With the deterministic software-pipelined emission
            # the prefetch/vT tiles only hold a slot transiently (drained a
            # couple of j-steps later), so sims keep >=2 slots at all times.
            simp = ctx.enter_context(
                tc.tile_pool(name="simp", bufs=3, space="PSUM"))
            auxp = simp
            poutp = ctx.enter_context(
                tc.tile_pool(name="poutp", bufs=1, space="PSUM"))

            # ---- persistent weights / constants (loaded once) ----
            wq = [pers.tile([128, QKV_DIM], bf16, name=f"NM_wq{t}") for t in range(4)]
            wo = [pers.tile([128, C], bf16, name=f"NM_wo{t}") for t in range(4)]
            gh_sb = [consts.tile([128, GROUPS], f32, name=f"NM_gh{t}") for t in range(4)]
            gt_sb = consts.tile([GROUPS, C], f32, name="gt", tag="gt")
            bq_sb = consts.tile([128, 8], f32, name="bq", tag="bq")
            bo_sb = consts.tile([128, 4], f32, name="bo", tag="bo")
            gam_sb = consts.tile([128, 4], f32, name="gam", tag="gam")
            bet_sb = consts.tile([128, 4], f32, name="bet", tag="bet")
            eb_sb = consts.tile([128, 1], f32, name="eb", tag="eb")
            magic = consts.tile([GROUPS, 1], mybir.dt.int32, name="magic",
                                tag="magic")
            nc.vector.memset(eb_sb, -EBIAS)
            nc.vector.memset(magic, 0x5f3759df)

            for t in range(4):
                nc.sync.dma_start(out=gh_sb[t], in_=gh_d[t * 128:(t + 1) * 128, :])
            nc.sync.dma_start(out=gt_sb, in_=gt_d)
            nc.sync.dma_start(out=bq_sb, in_=bq_d)
            nc.sync.dma_start(out=bo_sb, in_=bo_d)
            nc.sync.dma_start(out=gam_sb, in_=gam_d)
            nc.sync.dma_start(out=bet_sb, in_=bet_d)
            for t in range(4):
                nc.sync.dma_start(out=wq[t], in_=wq_d[t * 128:(t + 1) * 128, :])
            for t in range(4):
                nc.sync.dma_start(out=wo[t], in_=wo_d[t * 128:(t + 1) * 128, :])

            # One persistent stationary tile for the attention out-matmuls:
            # vtt[:, j, 256*p + :] = [vT_A | ones64 | ones64 | vT_B] for head
            # pair p (fp8e4 for DoubleRow). Ones columns loaded once; vT
            # columns written per iteration from the direct vT matmul.
            vtt = vtp.tile([128, 8, 1024], f8, name="vtt", tag="vtt", bufs=1)
            for p4 in range(4):
                base1 = vtt[:, 0, 256 * p4 + 64:256 * p4 + 65]
                dst1 = bass.AP(tensor=base1.tensor, offset=base1.offset,
                               ap=[base1.ap[0], [1024, 8], [1, 128]])
                nc.vector.memset(dst1, 1.0)

            def body(it, x_src, dst):
                xs = [pers.tile([128, N], f32, name=f"xs{t}_{it}", tag=f"xs{t}")
                      for t in range(4)]
                hs = [pers.tile([128, N], bf16, name=f"hs{t}_{it}", tag=f"hs{t}")
                      for t in range(4)]
                qkv = [pers.tile([128, N], bf16, name=f"qkv{m}_{it}", tag=f"qkv{m}")
                       for m in range(8)]
                att = [pers.tile([128, N], bf16, name=f"att{t}_{it}", tag=f"att{t}")
                      for t in range(4)]
                osb = [pers.tile([128, N], f32, name=f"osb{t}_{it}", tag=f"osb{t}")
                       for t in range(4)]
                sq_sb = consts.tile([128, 8], f32, name=f"sq_{it}", tag="sq")
                AB_sb = consts.tile([128, 8], f32, name=f"AB_{it}", tag="AB")
                musig = consts.tile([GROUPS, 2], f32, name=f"musig_{it}", tag="musig")

                for t in range(4):
                    nc.sync.dma_start(out=xs[t], in_=x_src[t * 128:(t + 1) * 128, :])

                # ---------------- GroupNorm ----------------
                gnp = simp
                # GN stats entirely off ACT (keeps the chained ACT stream a
                # pure exp pipeline with a single activation table): squares
                # on Pool, both reductions on DVE
                for t in range(4):
                    nc.vector.reduce_sum(out=sq_sb[:, 2 * t:2 * t + 1],
                                         in_=xs[t], axis=AX.X)
                    scr = scrp.tile([128, N], f32, name=f"scr_{it}_{t}",
                                    tag="scr")
                    nc.gpsimd.tensor_tensor(out=scr, in0=xs[t], in1=xs[t],
                                            op=OP.mult)
                    nc.vector.reduce_sum(out=sq_sb[:, 2 * t + 1:2 * t + 2],
                                         in_=scr, axis=AX.X)
                gstat = gnp.tile([GROUPS, 2], f32, name=f"gstat_{it}", tag="sim")
                for t in range(4):
                    nc.tensor.matmul(gstat, lhsT=gh_sb[t],
                                     rhs=sq_sb[:, 2 * t:2 * t + 2],
                                     start=(t == 0), stop=(t == 3))
                # musig[:,0] = mu ; musig[:,1] = rsqrt(var + eps)
                nc.vector.tensor_copy(out=musig[:, 0:1], in_=gstat[:, 0:1])
                musq = smalls.tile([GROUPS, 1], f32, name=f"musq_{it}", tag="musq")
                nc.vector.tensor_tensor(out=musq, in0=gstat[:, 0:1],
                                        in1=musig[:, 0:1], op=OP.mult)
                vpe = smalls.tile([GROUPS, 1], f32, name=f"vpe_{it}", tag="vpe")
                nc.vector.scalar_tensor_tensor(out=vpe, in0=gstat[:, 1:2],
                                               scalar=EPS, in1=musq,
                                               op0=OP.add, op1=OP.subtract)
                # rsqrt(var+eps) via quake seed + 2 Newton iterations (DVE
                # only; keeps the ACT table on the square+exp set all run)
                i32 = mybir.dt.int32
                vh = smalls.tile([GROUPS, 1], f32, name=f"vh_{it}", tag="vh")
                yr = smalls.tile([GROUPS, 1], f32, name=f"yr_{it}", tag="yr")
                t2 = smalls.tile([GROUPS, 1], f32, name=f"t2_{it}", tag="t2")
                t3 = smalls.tile([GROUPS, 1], f32, name=f"t3_{it}", tag="t3")
                nc.vector.tensor_scalar_mul(out=vh, in0=vpe, scalar1=0.5)
                nc.vector.tensor_scalar(out=t2.bitcast(i32),
                                        in0=vpe.bitcast(i32), scalar1=1,
                                        scalar2=None, op0=OP.arith_shift_right)
                nc.vector.tensor_tensor(out=yr.bitcast(i32), in0=magic,
                                        in1=t2.bitcast(i32), op=OP.subtract)
                for nwt in range(2):
                    nc.vector.tensor_tensor(out=t2, in0=yr, in1=yr, op=OP.mult)
                    nc.vector.tensor_tensor(out=t2, in0=t2, in1=vh, op=OP.mult)
                    nc.vector.tensor_scalar(out=t3, in0=t2, scalar1=-1.0,
                                            scalar2=1.5, op0=OP.mult, op1=OP.add)
                    dst_y = musig[:, 1:2] if nwt == 1 else yr
                    nc.vector.tensor_tensor(out=dst_y, in0=yr, in1=t3,
                                            op=OP.mult)
                for t in range(4):
                    bcs = gnp.tile([128, 2], f32, name=f"bcs_{it}_{t}", tag="sim")
                    nc.tensor.matmul(bcs, lhsT=gt_sb[:, t * 128:(t + 1) * 128],
                                     rhs=musig, start=True, stop=True)
                    # A = gamma * rsig_c ; B = beta - mu_c * A
                    nc.vector.tensor_tensor(out=AB_sb[:, 2 * t:2 * t + 1],
                                            in0=bcs[:, 1:2],
                                            in1=gam_sb[:, t:t + 1], op=OP.mult)
                    muA = smalls.tile([128, 1], f32, name=f"muA_{it}_{t}",
                                      tag="muA")
                    nc.vector.tensor_tensor(out=muA, in0=bcs[:, 0:1],
                                            in1=AB_sb[:, 2 * t:2 * t + 1],
                                            op=OP.mult)
                    nc.vector.tensor_tensor(out=AB_sb[:, 2 * t + 1:2 * t + 2],
                                            in0=bet_sb[:, t:t + 1], in1=muA,
                                            op=OP.subtract)
                    nc.gpsimd.tensor_scalar(out=hs[t], in0=xs[t],
                                            scalar1=AB_sb[:, 2 * t:2 * t + 1],
                                            scalar2=AB_sb[:, 2 * t + 1:2 * t + 2],
                                            op0=OP.mult, op1=OP.add)

                # ---------------- QKV + attention (per head pair) ----------------
                def qkv_chunk(m, deprio=False):
                    from contextlib import nullcontext
                    pool, tg = (auxp, "aux") if deprio else (simp, "sim")
                    with (tc.high_priority(offset=-50000) if deprio
                          else nullcontext()):
                        ps = pool.tile([128, N], f32, name=f"mmps_{it}_{m}",
                                       tag=tg)
                        for i2 in range(2):
                            isl = slice(i2 * 512, (i2 + 1) * 512)
                            for k in range(4):
                                nc.tensor.matmul(
                                    ps[:, isl],
                                    lhsT=wq[k][:, m * 128:(m + 1) * 128],
                                    rhs=hs[k][:, isl],
                                    start=(k == 0), stop=(k == 3))
                    # PSUM->SBUF drain at normal priority: a deprioritized
                    # drain would hold the psum slot and starve the sim stream
                    nc.vector.tensor_scalar_add(out=qkv[m], in0=ps,
                                                scalar1=bq_sb[:, m:m + 1])

                qkv_chunk(0)
                qkv_chunk(4)

                def mk_vt(j):
                    # direct vT chunk j for all heads (v bias is folded into
                    # the out-proj bias on the host)
                    jsl = slice(j * 128, (j + 1) * 128)
                    tps = auxp.tile([128, 512], f32, name=f"vtp_{it}_{j}",
                                    tag="sim")
                    for k in range(4):
                        nc.tensor.matmul(tps, lhsT=hs[k][:, jsl],
                                         rhs=wq[k][:, 1024:1536],
                                         start=(k == 0), stop=(k == 3))
                    vbase = vtt[:, j, 0:1]
                    vdst = bass.AP(tensor=vbase.tensor, offset=vbase.offset,
                                   ap=[vbase.ap[0], [256, 4], [192, 2],
                                       [1, 64]])
                    nc.vector.tensor_copy(
                        out=vdst,
                        in_=tps.rearrange("p (q h d) -> p q h d", q=4, h=2))

                def mk_prefetch(m):
                    # thunk list: 8 matmuls then the PSUM->SBUF bias drain
                    ps = auxp.tile([128, N], f32, name=f"mmps_{it}_{m}",
                                   tag="sim")
                    th = []
                    for i2 in range(2):
                        isl = slice(i2 * 512, (i2 + 1) * 512)
                        for k in range(4):
                            th.append(lambda m=m, isl=isl, k=k, ps=ps:
                                      nc.tensor.matmul(
                                          ps[:, isl],
                                          lhsT=wq[k][:, m * 128:(m + 1) * 128],
                                          rhs=hs[k][:, isl],
                                          start=(k == 0), stop=(k == 3)))
                    th.append(lambda m=m, ps=ps: nc.vector.tensor_scalar_add(
                        out=qkv[m], in0=ps, scalar1=bq_sb[:, m:m + 1]))
                    return th

                # Software-pipelined emission: every engine stream is executed
                # in (priority ~ emission) order, so the DR out-matmuls of
                # round r are deferred two j-steps (their exp input is done by
                # then -> no PE stall), the final round + softmax epilogue of
                # each head ride inside the NEXT head's stream, and the
                # qkv-prefetch / vT matmuls are dealt out a few per j-step so
                # the sim->exp cadence (the ACT bottleneck) never starves.
                pending = []
                for p in range(4):
                    qt, kt = qkv[p], qkv[4 + p]

                    # vtt[:, j, 256p:256(p+1)] = [vT_A | ones | ones | vT_B]:
                    # head A out-mm (cols 0:128) -> psum rows 0:64 = numerator,
                    # 64:128 = denominator; head B (cols 128:256) -> rows
                    # 0:64 = denominator, 64:128 = numerator.
                    for hh in range(2):
                        hsl = slice(hh * 64, (hh + 1) * 64)
                        csl = slice(256 * p + hh * 128, 256 * p + (hh + 1) * 128)
                        Et = ep.tile([128, 8, N], f8, name=f"E_{it}_{p}_{hh}",
                                     tag="E")
                        pout = poutp.tile([128, N], f32,
                                          name=f"pout_{it}_{p}_{hh}", tag="pout")

                        pf = []
                        if hh == 1 and p < 3:
                            pf = mk_prefetch(p + 1) + mk_prefetch(4 + p + 1)
                        # chunks of pf dealt per j-step (pair 0 starts at j2:
                        # the aux psum slot is busy with vT chunks 6/7 there)
                        if p == 0:
                            deal = [0, 0, 3, 3, 3, 3, 3, 3]
                        else:
                            deal = [3, 3, 2, 2, 2, 2, 2, 2]

                        def mk_outs(r, Et=Et, pout=pout, csl=csl):
                            j1 = 2 * r + 1
                            for i2 in range(2):
                                isl = slice(i2 * 512, (i2 + 1) * 512)
                                nc.tensor.matmul(
                                    pout[:, isl],
                                    lhsT=vtt[:, j1 - 1:j1 + 1, csl],
                                    rhs=Et[:, j1 - 1:j1 + 1, isl],
                                    perf_mode=DR,
                                    start=(r == 0), stop=(r == 3))

                        def mk_epi(hh=hh, p=p, pout=pout):
                            asl = slice(hh * 64, (hh + 1) * 64)
                            dsl = slice((1 - hh) * 64, (2 - hh) * 64)
                            rcp = smalls.tile([128, N], f32,
                                              name=f"rcp_{it}_{p}_{hh}",
                                              tag="rcp")
                            nc.vector.reciprocal(out=rcp[asl, :],
                                                 in_=pout[dsl, :])
                            nc.vector.tensor_tensor(
                                out=att[p][asl, :], in0=pout[asl, :],
                                in1=rcp[asl, :], op=OP.mult)

                        for j in range(8):
                            if j % 2 == 1:
                                for th in pending:
                                    th()
                                pending = []
                            jsl = slice(j * 128, (j + 1) * 128)
                            ps = simp.tile([128, N], f32,
                                           name=f"sps_{it}_{p}_{hh}_{j}",
                                           tag="sim")
                            for i2 in range(2):
                                isl = slice(i2 * 512, (i2 + 1) * 512)
                                nc.tensor.matmul(ps[:, isl],
                                                 lhsT=kt[hsl, jsl],
                                                 rhs=qt[hsl, isl],
                                                 start=True, stop=True)
                            nc.scalar.activation(out=Et[:, j, :], in_=ps,
                                                 func=AF.Exp, scale=SCALE,
                                                 bias=eb_sb)
                            if p == 0 and hh == 0 and j < 6:
                                mk_vt(j)
                            if p == 0 and hh == 1 and j == 0:
                                # chunks 6+7 both here: the deferred final
                                # out-matmuls of (0,0) flush at j==1 and read
                                # vtt[:, 6:8, :]
                                mk_vt(6)
                                mk_vt(7)
                            for _ in range(deal[j]):
                                if pf:
                                    pf.pop(0)()
                            if j % 2 == 1:
                                r = j // 2
                                pending.append(
                                    lambda r=r, mk=mk_outs: mk(r))
                                if r == 3:
                                    pending.append(mk_epi)
                for th in pending:
                    th()
                pending = []

                # ---------------- out projection + bias + residual ----------------
                for t in range(4):
                    ps = simp.tile([128, N], f32, name=f"prps_{it}_{t}", tag="sim")
                    for i2 in range(2):
                        isl = slice(i2 * 512, (i2 + 1) * 512)
                        for k in range(4):
                            nc.tensor.matmul(
                                ps[:, isl],
                                lhsT=wo[k][:, t * 128:(t + 1) * 128],
                                rhs=att[k][:, isl],
                                start=(k == 0), stop=(k == 3))
                    nc.vector.scalar_tensor_tensor(
                        out=osb[t], in0=ps,
                        scalar=bo_sb[:, t:t + 1], in1=xs[t],
                        op0=OP.add, op1=OP.add)
                    oeng = nc.gpsimd if t % 2 == 0 else nc.sync
                    oeng.dma_start(out=dst[t * 128:(t + 1) * 128, :],
                                   in_=osb[t])

            for it in range(chain):
                x_src = x_d if it == 0 else bounce
                dst = out_d if it == chain - 1 else bounce
                body(it, x_src, dst)

    nc.compile()
    return nc


def _get_nc(chain=1):
    key = ("nc", chain)
    if key not in _CACHE:
        _CACHE[key] = _build(chain)
    return _CACHE[key]


def _prep_inputs(x, gn_gamma, gn_beta, w_qkv, b_qkv, w_out, b_out):
    import ml_dtypes
    f = np.float32
    bf = ml_dtypes.bfloat16
    xr = np.ascontiguousarray(np.asarray(x).reshape(B, C, N).astype(f))
    wqT = np.ascontiguousarray(np.asarray(w_qkv).astype(f).T).astype(bf)  # [512, 1536]
    woT = np.ascontiguousarray(np.asarray(w_out).astype(f).T).astype(bf)  # [512, 512]
    bqT = np.ascontiguousarray(
        np.asarray(b_qkv).astype(f)[:1024].reshape(8, 128).T)
    # v-bias folds into the out-projection bias exactly (softmax rows sum
    # to 1): out = W_o(att_nb + b_v) + b_out = W_o att_nb + (b_out + W_o b_v)
    b_out_eff = (np.asarray(b_out).astype(np.float64) +
                 np.asarray(w_out).astype(np.float64)
                 @ np.asarray(b_qkv).astype(np.float64)[1024:1536]).astype(f)
    boT = np.ascontiguousarray(b_out_eff.reshape(4, 128).T)
    gamT = np.ascontiguousarray(np.asarray(gn_gamma).astype(f).reshape(4, 128).T)
    betT = np.ascontiguousarray(np.asarray(gn_beta).astype(f).reshape(4, 128).T)
    ch = np.arange(C)
    Ghat = np.zeros((C, GROUPS), f)
    Ghat[ch, ch // 16] = 1.0 / (16 * N)
    GT = np.zeros((GROUPS, C), f)
    GT[ch // 16, ch] = 1.0
    shared = dict(w_qkvT=wqT, w_outT=woT, b_qkvT=bqT, b_outT=boT,
                  gammaT=gamT, betaT=betT, Ghat=Ghat, GT=GT)
    return [dict(x=xr[i], **shared) for i in range(N_CORES)]


def _run(inputs, trace=False, trace_kwargs=None, chain=1):
    from concourse.bass_utils import run_bass_kernel_spmd
    nc = _get_nc(chain)
    in_maps = _prep_inputs(**inputs)
    res = run_bass_kernel_spmd(nc, in_maps, list(range(N_CORES)),
                               trace=trace, **(trace_kwargs or {}))
    out = np.stack([res.results[i]["out"] for i in range(N_CORES)])
    return out.reshape(B, C, 32, 32), res


def kernel(**inputs):
    out, _ = _run(inputs, trace=False)
    return out.astype(np.float32)
